# revision 66
# baseline (speedup 1.0000x reference)
"""Trainium2 Bass kernel for nn_DHT_Layer (conv1x1+BN+ReLU -> Deep Hough
Transform -> two 3x3 conv+BN+ReLU layers).

Sharding: data-parallel over batch. 8 images / 8 cores -> one image per
core, no collectives; full inputs in, full output out.  Everything heavy
runs as fp8e4 DoubleRow matmuls (0.5 PE cycles per output column -- 2x the
bf16 rate); the 2e-2 error budget is spent where quantization noise
averages out (the DHT's ~100-term positive sums) or is compensated by
hi+lo e4m3 splits (weights AND activations of the 3x3 convs).  Measured
rel err ~4.5e-3.  Per core:
  conv1 : runs TRANSPOSED, writing h1T[pixel, co] directly: per 100-pixel
          chunk one DoubleRow matmul with the x chunk as the STATIONARY
          operand (x host-permuted so each chunk is a [128ci, 2half, 100px]
          slab with the half stride 128 -- a walrus dual-fp8 ldweights
          requirement) and w1 moving.  The BN bias enters as a K=1
          DoubleRow product (ones x [bias_hi|bias_lo]).  No PE transposes,
          no psum drains, no intermediate h1: the ReLU*2^-4 epilogue goes
          straight from psum to fp8 h1T (ACT/DVE alternating per 8-chunk
          bank).  ~13k PE cyc.
  DHT   : out[c,a,r] = sum_p h[c,p] * (idx[a,p]==r) as fp8 one-hot
          DoubleRow matmuls.  Pixels are chunked 10x10 (100 chunks of 100
          pixels); per angle, chunks are greedily matched to minimize the
          rho-window union of each pair, and every pair runs as one
          DoubleRow matmul over the union window; leftovers pair against a
          zero chunk slot in h1T.  ~25k cyc-equivalents vs 90.7k bf16.
          The fp8 one-hots are precomputed on host (geometry-only) and
          streamed per 4-angle band via DMA (~14 MB, fully overlapped,
          ordered strictly behind the x stream on the shared DMA device).
  conv2/3: 3x3 convs with BOTH operands fp8: weights split hi+lo e4m3,
          activations split hi+lo e4m3 at drain time ([lo|hi] halves of
          one pad tile so DoubleRow pair strides stay positive).  The 27
          products per tap set (w_hi*x_hi + w_hi*x_lo + w_lo*x_hi; the
          ~0.07% lo*lo term is dropped) pack into 14 DoubleRow passes (the
          odd product pairs with a zero-weight block) over the FLATTENED
          zero-padded rows of 2-angle sub-bands (rhs stays 3-dim as the
          CoreSim DoubleRow executor requires; junk row-boundary columns
          land in psum and are skipped by PADW-strided epilogues).  7.0
          cyc/col vs 9.0 bf16.  conv2 lags the DHT drain by 2 bands and
          conv3 by 4, so PE never waits on an ACT drain it just requested.
Scales 2^-4 (DHT domain) and 2^-5 (conv3 input domain) keep fp8 in range;
they are exact powers of two folded into the next layer's weights.

The local walrus build only supports ONE sync-wait per instruction, so a
post-pass splits multi-wait instructions into single-wait NoOp carriers.
"""

import functools
import math

import ml_dtypes
import numpy as np

N = 8          # batch / cores
CIN = 256
CMID = 128
H = W = 100
HW = H * W
A = 100        # angles
R = 100        # rho bins
BH, BW = 10, 10
YBLK = 10
XBLK = 10
NBLK = YBLK * XBLK             # 100 chunks
BP = BH * BW                   # 100 pixels per chunk (contraction dim)
BAND = 4       # angles per PSUM bank / conv2 row band
NBAND = A // BAND
PADW = W + 2   # 102 padded spatial for 3x3 convs
BN_EPS = 1e-5
BF16 = ml_dtypes.bfloat16
FP8 = ml_dtypes.float8_e4m3
# power-of-two activation scales keep fp8 in range (DHT sums reach ~920,
# conv2+BN outputs ~1700); exact in fp8, inverses folded into w2/w3
SC1 = 2.0 ** -4     # h1T / DHT domain
SC2 = 2.0 ** -5     # h2 / conv3-input domain


# ----------------------------------------------------------------------------
# host-side precomputation (shapes are fixed -> cache)
# ----------------------------------------------------------------------------

def _hough_idx():
    irho = int(math.sqrt(H * H + W * W) + 1) / float(R)
    theta = np.arange(A) * (math.pi / A)
    tab_cos = np.cos(theta) / irho
    tab_sin = np.sin(theta) / irho
    yy, xx = np.meshgrid(np.arange(H) - H // 2, np.arange(W) - W // 2,
                         indexing='ij')
    r = np.round(xx[None, :, :] * tab_cos[:, None, None]
                 + yy[None, :, :] * tab_sin[:, None, None])
    return np.clip(r + R // 2, 0, R - 1).astype(np.int32)  # [A, H, W]


@functools.lru_cache(maxsize=1)
def _dht_plan():
    """Per-angle DoubleRow pairing plan + host-built fp8 one-hot table.

    Returns dict with:
      entries[a]: list of ('p', k1, k2, lo_u, win_u, coloff); k2 may be
                  NBLK (the zero chunk); coloff is absolute into ohtab.
      band_off[b]: first ohtab column of band b (b in 0..NBAND, sentinel).
      ohtab: [BP, TOT] fp8 one-hot table.
      ohmax: max columns of any band.
    """
    idx = _hough_idx()
    lo = np.zeros((A, NBLK), np.int64)
    hi = np.zeros((A, NBLK), np.int64)
    # pix[k, p] = (y, x) of partition p in chunk k
    sub_idx = np.zeros((A, NBLK, BP), np.int64)
    for gy in range(YBLK):
        for gx in range(XBLK):
            k = gy * XBLK + gx
            sub = idx[:, gy * BH:(gy + 1) * BH,
                      gx * BW:(gx + 1) * BW].reshape(A, BP)
            sub_idx[:, k] = sub
            lo[:, k] = sub.min(axis=1)
            hi[:, k] = sub.max(axis=1)
    win = hi - lo + 1

    def pairing(a):
        """Greedy min-union matching over lo-sorted neighbors; leftovers
        pair with the h1T zero block (chunk id NBLK) at 0.5x their window."""
        order = np.argsort(lo[a], kind='stable')
        cand = []
        for i in range(NBLK):
            for j in range(i + 1, min(i + 9, NBLK)):
                k1, k2 = int(order[i]), int(order[j])
                u = (max(hi[a, k1], hi[a, k2])
                     - min(lo[a, k1], lo[a, k2]) + 1)
                cand.append((u, k1, k2))
        cand.sort()
        used = np.zeros(NBLK, bool)
        pairs = []
        for u, k1, k2 in cand:
            if used[k1] or used[k2]:
                continue
            used[k1] = used[k2] = True
            pairs.append((k1, k2))
            if len(pairs) == NBLK // 2:
                break
        singles = [int(k) for k in np.where(~used)[0]]
        return pairs, singles

    entries = []
    band_off = [0]
    ohcols = []        # list of np arrays [BP] per column
    for b in range(NBAND):
        for s in range(BAND):
            a = b * BAND + s
            pairs, singles = pairing(a)
            # leftovers pair with the zero block (chunk NBLK); their second
            # one-hot member duplicates the first (zero weights kill it)
            jobs = ([(k1, k2, k1, k2) for k1, k2 in pairs]
                    + [(k, NBLK, k, k) for k in singles])
            ents = []
            for k1, k2, m1, m2 in jobs:
                lo_u = int(min(lo[a, m1], lo[a, m2]))
                win_u = int(max(hi[a, m1], hi[a, m2])) - lo_u + 1
                coloff = len(ohcols)
                for k in (m1, m2):
                    rel = sub_idx[a, k] - lo_u          # [BP]
                    oh = np.zeros((win_u, BP), np.float32)
                    oh[rel, np.arange(BP)] = 1.0
                    for j in range(win_u):
                        ohcols.append(oh[j])
                ents.append(('p', k1, k2, lo_u, win_u, coloff))
            entries.append(ents)
        band_off.append(len(ohcols))
    ohtab = np.ascontiguousarray(
        np.stack(ohcols, axis=1).astype(FP8))       # [BP, TOT]
    ohmax = max(band_off[i + 1] - band_off[i] for i in range(NBAND))
    return dict(entries=entries, band_off=band_off, ohtab=ohtab,
                ohmax=ohmax)


def _q8(x):
    return x.astype(FP8).astype(np.float32)


def _prep_weights(w1, b1, g1, be1, m1, v1, w2, b2, g2, be2, m2, v2,
                  w3, b3, g3, be3, m3, v3):
    s1 = g1 / np.sqrt(v1 + BN_EPS)
    s2 = g2 / np.sqrt(v2 + BN_EPS)
    s3 = g3 / np.sqrt(v3 + BN_EPS)
    # conv1: y[co] = sum_ci w1[co,ci]*x[ci]; fold BN scale into co rows.
    # single e4m3 (the quantization noise washes out in the DHT bins);
    # layout [ci%128, half*128 + co].
    w1f = (w1[:, :, 0, 0] * s1[:, None]).T            # [ci=256, co=128]
    w1h = w1f.reshape(2, 128, 128).transpose(1, 0, 2)  # [ci128, half, co]
    w1p8 = np.ascontiguousarray(w1h.reshape(128, 256).astype(FP8))
    # bias enters the conv1 psum as a K=1 DoubleRow product (ones x bias);
    # hi/lo fp8 rows keep it exact to ~0.07%.  [1, 2*8*128]: [bhi x8|blo x8]
    bias1 = ((b1 - m1) * s1 + be1).astype(np.float32)
    b_hi = _q8(bias1)
    b_lo = bias1 - b_hi
    bias18 = np.ascontiguousarray(np.concatenate(
        [np.tile(b_hi, 8), np.tile(b_lo, 8)]).reshape(1, 2048).astype(FP8))

    # conv2/3: fp8 hi/lo split, layout [ci, hi(9*128) | lo(9*128) | zero(128)]
    def conv_w8(w, s, inv_in_scale):
        wf = (w * s[:, None, None, None]).transpose(2, 3, 1, 0)  # [ky,kx,ci,co]
        wf = wf.reshape(9, 128, 128).transpose(1, 0, 2)          # [ci,tap,co]
        wf = wf * inv_in_scale
        w_hi = _q8(wf)
        w_lo = wf - w_hi
        # [zero(128) | hi(1152) | lo(1152)]
        arr = np.zeros((128, 2432), np.float32)
        arr[:, 128:1280] = w_hi.reshape(128, 1152)
        arr[:, 1280:2432] = w_lo.reshape(128, 1152)
        return np.ascontiguousarray(arr.astype(FP8))

    w2p8 = conv_w8(w2, s2, 1.0 / SC1)
    bias2 = (((b2 - m2) * s2 + be2) * SC2).astype(np.float32).reshape(128, 1)
    w3p8 = conv_w8(w3, s3, 1.0 / SC2)
    bias3 = ((b3 - m3) * s3 + be3).astype(np.float32).reshape(128, 1)
    return w1p8, bias18, w2p8, bias2, w3p8, bias3


# ----------------------------------------------------------------------------
# walrus workaround: split multi-wait instructions (this build supports only
# one sync-wait per instruction)
# ----------------------------------------------------------------------------

def _split_multi_waits(nc, mybir, max_waits=1):
    cnt = 0
    for f in nc.m.functions:
        for bb in f.blocks:
            insts = list(bb.instructions)
            new = []
            changed = False
            for inst in insts:
                si = inst.sync_info
                if si is not None:
                    ow = list(si.on_wait)
                    if len(ow) > max_waits:
                        changed = True
                        head = ow[:-max_waits]
                        for i in range(0, len(head), max_waits):
                            nop = mybir.InstNoOp(name=f'waitsplit_{cnt}',
                                                 ins=[], outs=[])
                            cnt += 1
                            nop.engine = inst.engine
                            nop.sync_info = mybir.SyncInfo(
                                on_wait=head[i:i + max_waits], on_update=[])
                            new.append(nop)
                        si.on_wait = ow[-max_waits:]
                new.append(inst)
            if changed:
                bb.instructions = new
    return cnt


# ----------------------------------------------------------------------------
# bass program
# ----------------------------------------------------------------------------

_PROGRAM_CACHE = {}


def _build_program(split_waits=True):
    key = ('nc', split_waits)
    if key in _PROGRAM_CACHE:
        return _PROGRAM_CACHE[key]
    import concourse.bass as bass
    import concourse.mybir as mybir
    import concourse.tile as tile
    from concourse.ap import AP
    from contextlib import ExitStack

    plan = _dht_plan()
    ENTRIES = plan['entries']
    BAND_OFF = plan['band_off']
    OHMAX = plan['ohmax']
    OHTOT = BAND_OFF[-1]

    f32 = mybir.dt.float32
    bf16 = mybir.dt.bfloat16
    fp8 = mybir.dt.float8e4
    RELU = mybir.ActivationFunctionType.Relu
    COPY = mybir.ActivationFunctionType.Copy
    DR = mybir.MatmulPerfMode.DoubleRow

    nc = bass.Bass('TRN2', target_bir_lowering=False, debug=False)
    # x arrives host-permuted: col = chunk*228 + half*128 + p (halves of a
    # chunk 128 apart -- walrus dual-fp8 ldweights needs the row-group
    # stride to be a multiple of 128; the 28-byte gaps are zero)
    x_d = nc.dram_tensor('x', [128, NBLK * 228], fp8, kind='ExternalInput')
    w1_d = nc.dram_tensor('w1p8', [128, 256], fp8, kind='ExternalInput')
    b1_d = nc.dram_tensor('bias18', [1, 2048], fp8, kind='ExternalInput')
    w2_d = nc.dram_tensor('w2p8', [128, 2432], fp8, kind='ExternalInput')
    b2_d = nc.dram_tensor('bias2', [128, 1], f32, kind='ExternalInput')
    w3_d = nc.dram_tensor('w3p8', [128, 2432], fp8, kind='ExternalInput')
    b3_d = nc.dram_tensor('bias3', [128, 1], f32, kind='ExternalInput')
    oh_d = nc.dram_tensor('ohtab', [BP, OHTOT], fp8, kind='ExternalInput')
    out_d = nc.dram_tensor('out', [128, HW], f32, kind='ExternalOutput')

    def dr_ap(base_ap, offset, istride, icount, nstride, ncount, parts):
        """3-dim AP [parts, icount, ncount] for DoubleRow operands."""
        return AP(base_ap.tensor, base_ap.offset + offset,
                  [[base_ap.ap[0][0], parts],
                   [istride, icount], [nstride, ncount]])

    with tile.TileContext(nc) as tc, ExitStack() as st0:
        consts = st0.enter_context(tc.tile_pool(name='consts', bufs=1))
        h1t_pool = st0.enter_context(tc.tile_pool(name='h1t', bufs=1))
        pad_pool = st0.enter_context(tc.tile_pool(name='pads', bufs=1))
        outb_pool = st0.enter_context(tc.tile_pool(name='outb', bufs=3))
        oh_pool = st0.enter_context(tc.tile_pool(name='oh', bufs=3))

        w1_t = consts.tile([128, 256], fp8, tag='w1')
        w2_t = consts.tile([128, 2432], fp8, tag='w2')
        w3_t = consts.tile([128, 2432], fp8, tag='w3')
        b1_t = consts.tile([1, 2048], fp8, tag='b1')
        b2_t = consts.tile([128, 1], f32, tag='b2')
        b3_t = consts.tile([128, 1], f32, tag='b3')
        nc.scalar.dma_start(out=b1_t[:], in_=b1_d.ap())
        zero_t = consts.tile([128, 512], bf16, tag='zeros')
        nc.vector.memset(zero_t[:], 0.0)

        # chunk slot NBLK is an all-zero block: leftover DHT singles pair
        # against it so every DHT matmul runs in DoubleRow mode
        h1T = h1t_pool.tile([128, (NBLK + 1) * 128], fp8, tag='h1T')
        nc.vector.memset(h1T[:, NBLK * 128:(NBLK + 1) * 128], 0.0)

        oh_tiles = {}

        def issue_oh(b, eng):
            t = oh_pool.tile([128, OHMAX], fp8, tag='oh', name=f'oh_{b}')
            cols = BAND_OFF[b + 1] - BAND_OFF[b]
            eng.dma_start(out=t[:BP, :cols],
                          in_=oh_d.ap()[0:BP, BAND_OFF[b]:BAND_OFF[b + 1]])
            oh_tiles[b] = t

        # conv inputs as fp8 hi/lo pairs in ONE tile each ([lo | hi] halves
        # so all DoubleRow pair strides stay positive); h2_pad keeps the
        # bf16 conv2 output so lo = relu(psum+b) - hi is computable.
        HOFF = PADW * PADW
        dpad8 = pad_pool.tile([128, 2 * HOFF], fp8, tag='dpad8')
        hpad8 = pad_pool.tile([128, 2 * HOFF], fp8, tag='hpad8')
        h2_pad = pad_pool.tile([128, PADW * PADW], bf16, tag='h2_pad')

        def pad_border_memsets():
            # zero only the borders; the interior is fully overwritten.
            for pad_t in (dpad8, hpad8):
                pv = pad_t[:].rearrange('c (two a r) -> c two a r', two=2,
                                        a=PADW)
                nc.gpsimd.memset(pv[:, :, 0:1, :], 0.0)
                nc.gpsimd.memset(pv[:, :, PADW - 1:PADW, :], 0.0)
                nc.gpsimd.memset(pv[:, :, :, 0:1], 0.0)
                nc.gpsimd.memset(pv[:, :, :, PADW - 1:PADW], 0.0)

        with ExitStack() as stT:
            # -------------------------------------------- transposed conv1
            # x arrives pixel-blocked ([ci, chunk*100+p]) so each chunk is a
            # contiguous 100-col slab.  conv1 runs TRANSPOSED: h1T[p, co] =
            # x_chunk^T @ w1 (x stationary, weights moving), writing h1T
            # directly -- no PE transposes, no psum drains, no h1 buffer.
            # Bias rides in as a K=1 DoubleRow product (ones x [bhi|blo]).
            with ExitStack() as st1:
                xf_pool = st1.enter_context(tc.tile_pool(name='xf', bufs=2))
                xb_pool = st1.enter_context(tc.tile_pool(name='xb', bufs=1))
                ps1 = st1.enter_context(
                    tc.tile_pool(name='ps1', bufs=3, space='PSUM'))

                GW = 10 * 228              # 2280: one block-row of chunks
                xbig = xb_pool.tile([128, 8 * GW], fp8, tag='xbig')
                ones_t = consts.tile([1, 256], fp8, tag='ones')
                nc.vector.memset(ones_t[:], 1.0)

                xf_tiles = {}

                def x_lhsT(k):
                    """stationary x chunk [K=128, 2(half), 100(pix)]."""
                    g = k // 10
                    kk = k % 10
                    if g < 2:
                        xa = xf_tiles[g][:]
                        off = kk * 228
                    else:
                        xa = xbig[:]
                        off = (g - 2) * GW + kk * 228
                    return AP(xa.tensor, xa.offset + off,
                              [[xa.ap[0][0], 128], [128, 2], [1, 100]])

                def conv1_bank(b):
                    """8 chunks (4 for the tail bank) -> h1T fp8."""
                    k0 = b * 8
                    kc = min(8, NBLK - k0)
                    ps = ps1.tile([128, 1024], f32, tag='ps1',
                                  name=f'c1_{b}')
                    ba = b1_t[:]
                    oa = ones_t[:]
                    # bias init: one K=1 DoubleRow matmul per 512-col zero
                    # region (adds bhi + blo exactly)
                    for rg in range((kc + 3) // 4):
                        n = min(512, kc * 128 - rg * 512)
                        lhsT = AP(oa.tensor, oa.offset,
                                  [[oa.ap[0][0], 1], [128, 2], [1, 128]])
                        rhs = AP(ba.tensor, ba.offset + rg * 512,
                                 [[ba.ap[0][0], 1], [1024, 2], [1, n]])
                        nc.tensor.matmul(out=ps[:, rg * 512:rg * 512 + n],
                                         lhsT=lhsT, rhs=rhs, start=True,
                                         stop=False, perf_mode=DR,
                                         skip_group_check=True)
                    for kk in range(kc):
                        k = k0 + kk
                        rhs = dr_ap(w1_t[:], 0, 128, 2, 1, 128, 128)
                        nc.tensor.matmul(
                            out=ps[:BP, kk * 128:(kk + 1) * 128],
                            lhsT=x_lhsT(k), rhs=rhs, start=False,
                            stop=(kk in (3, kc - 1)), perf_mode=DR,
                            skip_group_check=True)
                    # epilogue: relu * SC1, psum -> h1T fp8
                    dst = h1T[:BP, k0 * 128:(k0 + kc) * 128]
                    src = ps[:BP, :kc * 128]
                    if b % 2 == 0:
                        nc.scalar.activation(out=dst, in_=src, func=RELU,
                                             scale=SC1)
                    else:
                        nc.vector.tensor_scalar(
                            out=dst, in0=src, scalar1=0.0, scalar2=SC1,
                            op0=mybir.AluOpType.max,
                            op1=mybir.AluOpType.mult)

                for g in range(2):
                    xf = xf_pool.tile([128, GW], fp8, tag='xf',
                                      name=f'xf_{g}')
                    xf_tiles[g] = xf
                    (nc.sync, nc.gpsimd)[g].dma_start(
                        out=xf[:], in_=x_d.ap()[:, g * GW:(g + 1) * GW])
                    if g == 0:
                        nc.sync.dma_start(out=w1_t[:], in_=w1_d.ap())
                    if g == 1:
                        # groups 2-9 as 2-group DMAs (balances HWDGE fixed
                        # cost against DMA-device hold time)
                        for gg in range(4):
                            (nc.sync, nc.gpsimd)[gg % 2].dma_start(
                                out=xbig[:, gg * 2 * GW:(gg + 1) * 2 * GW],
                                in_=x_d.ap()[:, (2 + 2 * gg) * GW:
                                             (4 + 2 * gg) * GW])
                # one-hot bands 0-2 + conv weights on the SYNC queue so
                # their HWDGE slots (and thus DMA-device FIFO positions)
                # fall BEHIND every x transfer
                issue_oh(0, nc.sync)
                issue_oh(1, nc.sync)
                issue_oh(2, nc.sync)
                nc.sync.dma_start(out=w2_t[:], in_=w2_d.ap())
                nc.sync.dma_start(out=w3_t[:], in_=w3_d.ap())
                for b in range(13):
                    conv1_bank(b)

            pad_border_memsets()
            nc.scalar.dma_start(out=b2_t[:], in_=b2_d.ap())
            nc.scalar.dma_start(out=b3_t[:], in_=b3_d.ap())

            psd = stT.enter_context(
                tc.tile_pool(name='psd', bufs=4, space='PSUM'))
            psc = stT.enter_context(
                tc.tile_pool(name='psc', bufs=2, space='PSUM'))

            # -------------------------------------------------- DHT + convs
            def zero_bank(b, bank):
                # initialize the accumulator: all DHT matmuls use start=False
                # and accumulate onto zeroed PSUM.
                if b < 1:
                    nc.tensor.matmul(out=bank[:, :BAND * R],
                                     lhsT=zero_t[:1, :128],
                                     rhs=zero_t[:1, :BAND * R], start=True,
                                     stop=False, skip_group_check=True)
                elif b == 1:
                    nc.scalar.activation(out=bank[:, :BAND * R],
                                         in_=zero_t[:, :BAND * R],
                                         func=COPY)
                else:
                    nc.vector.memset(bank[:, :BAND * R], 0.0)

            def dht_band(b, bank):
                oh = oh_tiles[b]
                off0 = BAND_OFF[b]
                for s in range(BAND):
                    a = b * BAND + s
                    for _, k1, k2, lo_u, win_u, coloff in ENTRIES[a]:
                        lhsT = dr_ap(h1T[:], k1 * 128,
                                     (k2 - k1) * 128, 2, 1, 128, BP)
                        rhs = dr_ap(oh[:], coloff - off0,
                                    win_u, 2, 1, win_u, BP)
                        nc.tensor.matmul(
                            out=bank[:, s * R + lo_u:
                                     s * R + lo_u + win_u],
                            lhsT=lhsT, rhs=rhs, start=False, stop=False,
                            skip_group_check=True, perf_mode=DR)

            def pad_views(pad_t, a0, na):
                """(lo, hi) interior views [c, na, R] of a hi/lo pad tile."""
                pv = pad_t[:].rearrange('c (two a r) -> c two a r', two=2,
                                        a=PADW)
                lo = pv[:, 0:1, a0 + 1:a0 + 1 + na, 1:1 + R].squeeze(1)
                hi = pv[:, 1:2, a0 + 1:a0 + 1 + na, 1:1 + R].squeeze(1)
                return lo, hi

            def drain_band(b, bank):
                # psum -> fp8 hi (ACT) + fp8 lo residual (DVE subtract)
                a0 = b * BAND
                pv = bank[:, :BAND * R].rearrange('p (a r) -> p a r', a=BAND)
                lo_v, hi_v = pad_views(dpad8, a0, BAND)
                nc.scalar.activation(out=hi_v, in_=pv[:], func=COPY)
                nc.vector.tensor_tensor(out=lo_v, in0=pv[:], in1=hi_v,
                                        op=mybir.AluOpType.subtract)

            # conv psum layout: per band one [128, 1024] tile = 2 zero
            # regions; sub-band s2 (2 angles) accumulates over the FLATTENED
            # padded rows at cols [s2*512, s2*512+202) (junk at row-boundary
            # cols, skipped by the epilogue's PADW-strided reads).
            LSUB = PADW + R    # 202

            def emit14(ps, s2, pad_t, w_t, a0, lsub=None):
                """3-product fp8 conv: 14 DoubleRow passes accumulating
                w_hi*x_hi + w_hi*x_lo + w_lo*x_hi over 9 taps."""
                def wc(g, t):
                    return 128 + g * 1152 + t * 128

                def xc(h, t):
                    return (h * HOFF + (t // 3) * PADW + (t % 3)
                            + a0 * PADW)

                pairs = []
                for t in (0, 2, 4, 6):
                    pairs.append(((wc(0, t), xc(1, t)),
                                  (wc(0, t + 1), xc(1, t + 1))))
                for t in (0, 2, 4, 6):
                    pairs.append(((wc(0, t), xc(0, t)),
                                  (wc(0, t + 1), xc(0, t + 1))))
                for t in (0, 2, 4, 6):
                    pairs.append(((wc(1, t), xc(1, t)),
                                  (wc(1, t + 1), xc(1, t + 1))))
                pairs.append(((wc(0, 8), xc(0, 8)), (wc(1, 8), xc(1, 8))))
                # odd 27th product pairs with the zero-weight block (w col
                # 0); the dummy rhs member re-reads tap-0's valid window
                pairs.append(((0, xc(1, 0)), (wc(0, 8), xc(1, 8))))
                wa = w_t[:]
                xa = pad_t[:]
                if lsub is None:
                    lsub = LSUB
                out_v = ps[:, s2 * 512:s2 * 512 + lsub]
                for i, ((w1c, x1c), (w2c, x2c)) in enumerate(pairs):
                    lhsT = AP(wa.tensor, wa.offset + w1c,
                              [[wa.ap[0][0], 128], [w2c - w1c, 2], [1, 128]])
                    rhs = AP(xa.tensor, xa.offset + x1c,
                             [[xa.ap[0][0], 128], [x2c - x1c, 2], [1, lsub]])
                    nc.tensor.matmul(out=out_v, lhsT=lhsT, rhs=rhs,
                                     start=(i == 0),
                                     stop=(i == len(pairs) - 1),
                                     perf_mode=DR)

            def conv_psum_view(ps, ar, sub=0):
                """[p, (sub, al), R] view of the padded conv psum."""
                pa = ps[:]
                if ar == BAND:
                    return AP(pa.tensor, pa.offset,
                              [[pa.ap[0][0], 128], [512, 2], [PADW, 2],
                               [1, R]])
                if ar == 1:
                    return AP(pa.tensor, pa.offset + sub * 512,
                              [[pa.ap[0][0], 128], [1, R]])
                return AP(pa.tensor, pa.offset,
                          [[pa.ap[0][0], 128], [PADW, ar], [1, R]])

            def conv2_band(c):
                a0 = c * BAND
                ps = psc.tile([128, 1024], f32, tag='conv')
                for s2 in range(2):
                    emit14(ps, s2, dpad8, w2_t, a0 + s2 * 2)
                pv = conv_psum_view(ps, BAND)
                hv2 = h2_pad[:].rearrange('c (a r) -> c a r', a=PADW)
                h2v = hv2[:, a0 + 1:a0 + 1 + BAND, 1:1 + R]
                h2v4 = h2v.rearrange('c (x y) r -> c x y r', x=2)
                nc.scalar.activation(out=h2v4, in_=pv, func=RELU,
                                     bias=b2_t[:, :1], scale=SC2)
                lo_v, hi_v = pad_views(hpad8, a0, BAND)
                nc.scalar.activation(out=hi_v, in_=h2v, func=COPY)
                nc.vector.tensor_tensor(out=lo_v, in0=h2v, in1=hi_v,
                                        op=mybir.AluOpType.subtract)

            def conv3_band(c, ar=BAND, s0=0, dve_epi=False, ps=None, sub=0):
                a0 = c * BAND + s0
                if ps is None:
                    ps = psc.tile([128, 1024], f32, tag='conv')
                if ar == 1:
                    emit14(ps, sub, hpad8, w3_t, a0, lsub=R)
                else:
                    for s2 in range(ar // 2):
                        emit14(ps, s2, hpad8, w3_t, a0 + s2 * 2)
                pv = conv_psum_view(ps, ar, sub)
                ob = outb_pool.tile([128, ar * R], f32, tag=f'outb{ar}')
                if ar == BAND:
                    ov = ob[:].rearrange('p (x y r) -> p x y r', x=2, y=2)
                elif ar == 1:
                    ov = ob[:]
                else:
                    ov = ob[:].rearrange('p (a r) -> p a r', a=ar)
                if dve_epi:
                    nc.vector.tensor_scalar(
                        out=ov, in0=pv, scalar1=b3_t[:, :1],
                        scalar2=0.0, op0=mybir.AluOpType.add,
                        op1=mybir.AluOpType.max)
                else:
                    nc.scalar.activation(out=ov, in_=pv, func=RELU,
                                         bias=b3_t[:, :1], scale=1.0)
                nc.sync.dma_start(out=out_d.ap()[:, a0 * R:(a0 + ar) * R],
                                  in_=ob[:])

            banks = {0: psd.tile([128, 512], f32, tag='band',
                                 name='bank_0')}
            zero_bank(0, banks[0])
            for b in range(NBAND):
                # stream the upcoming one-hot bands behind the PE
                if b + 3 < NBAND:
                    issue_oh(b + 3, (nc.sync, nc.gpsimd, nc.scalar)[b % 3])
                if b + 1 < NBAND:
                    banks[b + 1] = psd.tile([128, 512], f32, tag='band',
                                            name=f'bank_{b + 1}')
                    zero_bank(b + 1, banks[b + 1])
                dht_band(b, banks[b])
                drain_band(b, banks[b])
                del banks[b]
                del oh_tiles[b]
                if b >= 2:
                    conv2_band(b - 2)
                if b >= 4:
                    conv3_band(b - 4)
            for c in (NBAND - 2, NBAND - 1):
                conv2_band(c)
            for c in range(NBAND - 4, NBAND - 1):
                conv3_band(c)
            # split the last band so its epilogue/DMA pipeline with the
            # later pieces' matmuls instead of trailing the whole kernel
            conv3_band(NBAND - 1, ar=2, s0=0)
            ps_tail = psc.tile([128, 1024], f32, tag='conv')
            conv3_band(NBAND - 1, ar=1, s0=2, ps=ps_tail, sub=0)
            conv3_band(NBAND - 1, ar=1, s0=3, dve_epi=True, ps=ps_tail,
                       sub=1)

    if split_waits:
        _split_multi_waits(nc, mybir)
    _PROGRAM_CACHE[key] = nc
    return nc


# ----------------------------------------------------------------------------
# entry point
# ----------------------------------------------------------------------------

def make_in_maps(inputs):
    plan = _dht_plan()
    x = np.asarray(inputs['x'], np.float32)
    w1p8, bias18, w2p8, bias2, w3p8, bias3 = _prep_weights(
        *[np.asarray(inputs[k], np.float32) for k in
          ('w1', 'b1', 'g1', 'be1', 'm1', 'v1',
           'w2', 'b2', 'g2', 'be2', 'm2', 'v2',
           'w3', 'b3', 'g3', 'be3', 'm3', 'v3')])
    common = dict(w1p8=w1p8, bias18=bias18, w2p8=w2p8, bias2=bias2,
                  w3p8=w3p8, bias3=bias3, ohtab=plan['ohtab'])
    # x host-permuted: [ci%128, chunk*228 + (ci//128)*128 + p] with
    # p = dy*10+dx, chunk = gy*10+gx; 28-byte zero gaps per chunk
    xb = (x.reshape(N, 2, 128, YBLK, BH, XBLK, BW)
          .transpose(0, 2, 3, 5, 1, 4, 6)      # n, ci128, gy, gx, half, dy, dx
          .reshape(N, 128, NBLK, 2, BP))
    xp = np.zeros((N, 128, NBLK, 228), FP8)
    xp[:, :, :, 0:100] = xb[:, :, :, 0].astype(FP8)
    xp[:, :, :, 128:228] = xb[:, :, :, 1].astype(FP8)
    xp = xp.reshape(N, 128, NBLK * 228)
    return [
        {'x': np.ascontiguousarray(xp[n]), **common}
        for n in range(N)
    ]


def run(inputs, trace=False):
    from concourse.bass_utils import run_bass_kernel_spmd

    nc = _build_program()
    in_maps = make_in_maps(inputs)
    res = run_bass_kernel_spmd(nc, in_maps, core_ids=list(range(N)),
                               trace=trace)
    out = np.stack([res.results[n]['out'].reshape(CMID, H, W)
                    for n in range(N)], axis=0)
    return out.astype(np.float32), res


def kernel(**inputs):
    out, _ = run(inputs, trace=False)
    return out


# revision 74
# speedup vs baseline: 1.0151x; 1.0151x over previous
"""Trainium2 Bass kernel for nn_DHT_Layer (conv1x1+BN+ReLU -> Deep Hough
Transform -> two 3x3 conv+BN+ReLU layers).

Sharding: data-parallel over batch. 8 images / 8 cores -> one image per
core, no collectives; full inputs in, full output out.  Everything heavy
runs as fp8e4 DoubleRow matmuls (0.5 PE cycles per output column -- 2x the
bf16 rate); the 2e-2 error budget is spent where quantization noise
averages out (the DHT's ~100-term positive sums) or is compensated by
hi+lo e4m3 splits (weights AND activations of the 3x3 convs).  Measured
rel err ~4.5e-3.  Per core:
  conv1 : runs TRANSPOSED, writing h1T[pixel, co] directly: per 100-pixel
          chunk one DoubleRow matmul with the x chunk as the STATIONARY
          operand (x host-permuted so each chunk is a [128ci, 2half, 100px]
          slab with the half stride 128 -- a walrus dual-fp8 ldweights
          requirement) and w1 moving.  The BN bias enters as a K=1
          DoubleRow product (ones x [bias_hi|bias_lo]).  No PE transposes,
          no psum drains, no intermediate h1: the ReLU*2^-4 epilogue goes
          straight from psum to fp8 h1T (ACT/DVE alternating per 8-chunk
          bank).  ~13k PE cyc.
  DHT   : out[c,a,r] = sum_p h[c,p] * (idx[a,p]==r) as fp8 one-hot
          DoubleRow matmuls.  Pixels are chunked 10x10 (100 chunks of 100
          pixels); per angle, chunks are greedily matched to minimize the
          rho-window union of each pair, and every pair runs as one
          DoubleRow matmul over the union window; leftovers pair against a
          zero chunk slot in h1T.  ~25k cyc-equivalents vs 90.7k bf16.
          The fp8 one-hots are precomputed on host (geometry-only) and
          streamed per 4-angle band via DMA (~14 MB, fully overlapped,
          ordered strictly behind the x stream on the shared DMA device).
  conv2/3: 3x3 convs with BOTH operands fp8: weights split hi+lo e4m3,
          activations split hi+lo e4m3 at drain time ([lo|hi] halves of
          one pad tile so DoubleRow pair strides stay positive).  The 27
          products per tap set (w_hi*x_hi + w_hi*x_lo + w_lo*x_hi; the
          ~0.07% lo*lo term is dropped) pack into 14 DoubleRow passes (the
          odd product pairs with a zero-weight block) over the FLATTENED
          zero-padded rows of 2-angle sub-bands (rhs stays 3-dim as the
          CoreSim DoubleRow executor requires; junk row-boundary columns
          land in psum and are skipped by PADW-strided epilogues).  7.0
          cyc/col vs 9.0 bf16.  conv2 lags the DHT drain by 2 bands and
          conv3 by 4, so PE never waits on an ACT drain it just requested.
Scales 2^-4 (DHT domain) and 2^-5 (conv3 input domain) keep fp8 in range;
they are exact powers of two folded into the next layer's weights.

The local walrus build only supports ONE sync-wait per instruction, so a
post-pass splits multi-wait instructions into single-wait NoOp carriers.
"""

import functools
import math

import ml_dtypes
import numpy as np

N = 8          # batch / cores
CIN = 256
CMID = 128
H = W = 100
HW = H * W
A = 100        # angles
R = 100        # rho bins
BH, BW = 10, 10
YBLK = 10
XBLK = 10
NBLK = YBLK * XBLK             # 100 chunks
BP = BH * BW                   # 100 pixels per chunk (contraction dim)
BAND = 4       # angles per PSUM bank / conv2 row band
NBAND = A // BAND
PADW = W + 2   # 102 padded spatial for 3x3 convs
BN_EPS = 1e-5
BF16 = ml_dtypes.bfloat16
FP8 = ml_dtypes.float8_e4m3
# power-of-two activation scales keep fp8 in range (DHT sums reach ~920,
# conv2+BN outputs ~1700); exact in fp8, inverses folded into w2/w3
SC1 = 2.0 ** -4     # h1T / DHT domain
SC2 = 2.0 ** -5     # h2 / conv3-input domain


# ----------------------------------------------------------------------------
# host-side precomputation (shapes are fixed -> cache)
# ----------------------------------------------------------------------------

def _hough_idx():
    irho = int(math.sqrt(H * H + W * W) + 1) / float(R)
    theta = np.arange(A) * (math.pi / A)
    tab_cos = np.cos(theta) / irho
    tab_sin = np.sin(theta) / irho
    yy, xx = np.meshgrid(np.arange(H) - H // 2, np.arange(W) - W // 2,
                         indexing='ij')
    r = np.round(xx[None, :, :] * tab_cos[:, None, None]
                 + yy[None, :, :] * tab_sin[:, None, None])
    return np.clip(r + R // 2, 0, R - 1).astype(np.int32)  # [A, H, W]


@functools.lru_cache(maxsize=1)
def _dht_plan():
    """Per-angle DoubleRow pairing plan + host-built fp8 one-hot table.

    Returns dict with:
      entries[a]: list of ('p', k1, k2, lo_u, win_u, coloff); k2 may be
                  NBLK (the zero chunk); coloff is absolute into ohtab.
      band_off[b]: first ohtab column of band b (b in 0..NBAND, sentinel).
      ohtab: [BP, TOT] fp8 one-hot table.
      ohmax: max columns of any band.
    """
    idx = _hough_idx()
    lo = np.zeros((A, NBLK), np.int64)
    hi = np.zeros((A, NBLK), np.int64)
    # pix[k, p] = (y, x) of partition p in chunk k
    sub_idx = np.zeros((A, NBLK, BP), np.int64)
    for gy in range(YBLK):
        for gx in range(XBLK):
            k = gy * XBLK + gx
            sub = idx[:, gy * BH:(gy + 1) * BH,
                      gx * BW:(gx + 1) * BW].reshape(A, BP)
            sub_idx[:, k] = sub
            lo[:, k] = sub.min(axis=1)
            hi[:, k] = sub.max(axis=1)
    win = hi - lo + 1

    def pairing(a):
        """Greedy min-union matching over lo-sorted neighbors; leftovers
        pair with the h1T zero block (chunk id NBLK) at 0.5x their window."""
        order = np.argsort(lo[a], kind='stable')
        cand = []
        for i in range(NBLK):
            for j in range(i + 1, min(i + 9, NBLK)):
                k1, k2 = int(order[i]), int(order[j])
                u = (max(hi[a, k1], hi[a, k2])
                     - min(lo[a, k1], lo[a, k2]) + 1)
                cand.append((u, k1, k2))
        cand.sort()
        used = np.zeros(NBLK, bool)
        pairs = []
        for u, k1, k2 in cand:
            if used[k1] or used[k2]:
                continue
            used[k1] = used[k2] = True
            pairs.append((k1, k2))
            if len(pairs) == NBLK // 2:
                break
        singles = [int(k) for k in np.where(~used)[0]]
        return pairs, singles

    entries = []
    band_off = [0]
    ohcols = []        # list of np arrays [BP] per column
    for b in range(NBAND):
        for s in range(BAND):
            a = b * BAND + s
            pairs, singles = pairing(a)
            # leftovers pair with the zero block (chunk NBLK); their second
            # one-hot member duplicates the first (zero weights kill it)
            jobs = ([(k1, k2, k1, k2) for k1, k2 in pairs]
                    + [(k, NBLK, k, k) for k in singles])
            ents = []
            for k1, k2, m1, m2 in jobs:
                lo_u = int(min(lo[a, m1], lo[a, m2]))
                win_u = int(max(hi[a, m1], hi[a, m2])) - lo_u + 1
                coloff = len(ohcols)
                for k in (m1, m2):
                    rel = sub_idx[a, k] - lo_u          # [BP]
                    oh = np.zeros((win_u, BP), np.float32)
                    oh[rel, np.arange(BP)] = 1.0
                    for j in range(win_u):
                        ohcols.append(oh[j])
                ents.append(('p', k1, k2, lo_u, win_u, coloff))
            entries.append(ents)
        band_off.append(len(ohcols))
    ohtab = np.ascontiguousarray(
        np.stack(ohcols, axis=1).astype(FP8))       # [BP, TOT]
    ohmax = max(band_off[i + 1] - band_off[i] for i in range(NBAND))
    return dict(entries=entries, band_off=band_off, ohtab=ohtab,
                ohmax=ohmax)


def _q8(x):
    return x.astype(FP8).astype(np.float32)


def _prep_weights(w1, b1, g1, be1, m1, v1, w2, b2, g2, be2, m2, v2,
                  w3, b3, g3, be3, m3, v3):
    s1 = g1 / np.sqrt(v1 + BN_EPS)
    s2 = g2 / np.sqrt(v2 + BN_EPS)
    s3 = g3 / np.sqrt(v3 + BN_EPS)
    # conv1: y[co] = sum_ci w1[co,ci]*x[ci]; fold BN scale into co rows.
    # single e4m3 (the quantization noise washes out in the DHT bins);
    # layout [ci%128, half*128 + co].
    w1f = (w1[:, :, 0, 0] * s1[:, None]).T            # [ci=256, co=128]
    w1h = w1f.reshape(2, 128, 128).transpose(1, 0, 2)  # [ci128, half, co]
    w1p8 = np.ascontiguousarray(w1h.reshape(128, 256).astype(FP8))
    # bias enters the conv1 psum as a K=1 DoubleRow product (ones x bias);
    # hi/lo fp8 rows keep it exact to ~0.07%.  [1, 2*8*128]: [bhi x8|blo x8]
    bias1 = ((b1 - m1) * s1 + be1).astype(np.float32)
    b_hi = _q8(bias1)
    b_lo = bias1 - b_hi
    bias18 = np.ascontiguousarray(np.concatenate(
        [np.tile(b_hi, 8), np.tile(b_lo, 8)]).reshape(1, 2048).astype(FP8))

    # conv2/3: fp8 hi/lo split, layout [ci, hi(9*128) | lo(9*128) | zero(128)]
    def conv_w8(w, s, inv_in_scale):
        wf = (w * s[:, None, None, None]).transpose(2, 3, 1, 0)  # [ky,kx,ci,co]
        wf = wf.reshape(9, 128, 128).transpose(1, 0, 2)          # [ci,tap,co]
        wf = wf * inv_in_scale
        w_hi = _q8(wf)
        w_lo = wf - w_hi
        # [zero(128) | hi(1152) | lo(1152)]
        arr = np.zeros((128, 2432), np.float32)
        arr[:, 128:1280] = w_hi.reshape(128, 1152)
        arr[:, 1280:2432] = w_lo.reshape(128, 1152)
        return np.ascontiguousarray(arr.astype(FP8))

    w2p8 = conv_w8(w2, s2, 1.0 / SC1)
    bias2 = (((b2 - m2) * s2 + be2) * SC2).astype(np.float32).reshape(128, 1)
    w3p8 = conv_w8(w3, s3, 1.0 / SC2)
    bias3 = ((b3 - m3) * s3 + be3).astype(np.float32).reshape(128, 1)
    return w1p8, bias18, w2p8, bias2, w3p8, bias3


# ----------------------------------------------------------------------------
# walrus workaround: split multi-wait instructions (this build supports only
# one sync-wait per instruction)
# ----------------------------------------------------------------------------

def _split_multi_waits(nc, mybir, max_waits=1):
    cnt = 0
    for f in nc.m.functions:
        for bb in f.blocks:
            insts = list(bb.instructions)
            new = []
            changed = False
            for inst in insts:
                si = inst.sync_info
                if si is not None:
                    ow = list(si.on_wait)
                    if len(ow) > max_waits:
                        changed = True
                        head = ow[:-max_waits]
                        for i in range(0, len(head), max_waits):
                            nop = mybir.InstNoOp(name=f'waitsplit_{cnt}',
                                                 ins=[], outs=[])
                            cnt += 1
                            nop.engine = inst.engine
                            nop.sync_info = mybir.SyncInfo(
                                on_wait=head[i:i + max_waits], on_update=[])
                            new.append(nop)
                        si.on_wait = ow[-max_waits:]
                new.append(inst)
            if changed:
                bb.instructions = new
    return cnt


# ----------------------------------------------------------------------------
# bass program
# ----------------------------------------------------------------------------

_PROGRAM_CACHE = {}


def _build_program(split_waits=True):
    key = ('nc', split_waits)
    if key in _PROGRAM_CACHE:
        return _PROGRAM_CACHE[key]
    import concourse.bass as bass
    import concourse.mybir as mybir
    import concourse.tile as tile
    from concourse.ap import AP
    from contextlib import ExitStack

    plan = _dht_plan()
    ENTRIES = plan['entries']
    BAND_OFF = plan['band_off']
    OHMAX = plan['ohmax']
    OHTOT = BAND_OFF[-1]

    f32 = mybir.dt.float32
    bf16 = mybir.dt.bfloat16
    fp8 = mybir.dt.float8e4
    RELU = mybir.ActivationFunctionType.Relu
    COPY = mybir.ActivationFunctionType.Copy
    DR = mybir.MatmulPerfMode.DoubleRow

    nc = bass.Bass('TRN2', target_bir_lowering=False, debug=False)
    # x arrives host-permuted: col = chunk*228 + half*128 + p (halves of a
    # chunk 128 apart -- walrus dual-fp8 ldweights needs the row-group
    # stride to be a multiple of 128; the 28-byte gaps are zero)
    x_d = nc.dram_tensor('x', [128, NBLK * 228], fp8, kind='ExternalInput')
    w1_d = nc.dram_tensor('w1p8', [128, 256], fp8, kind='ExternalInput')
    b1_d = nc.dram_tensor('bias18', [1, 2048], fp8, kind='ExternalInput')
    w2_d = nc.dram_tensor('w2p8', [128, 2432], fp8, kind='ExternalInput')
    b2_d = nc.dram_tensor('bias2', [128, 1], f32, kind='ExternalInput')
    w3_d = nc.dram_tensor('w3p8', [128, 2432], fp8, kind='ExternalInput')
    b3_d = nc.dram_tensor('bias3', [128, 1], f32, kind='ExternalInput')
    oh_d = nc.dram_tensor('ohtab', [BP, OHTOT], fp8, kind='ExternalInput')
    out_d = nc.dram_tensor('out', [128, HW], f32, kind='ExternalOutput')

    def dr_ap(base_ap, offset, istride, icount, nstride, ncount, parts):
        """3-dim AP [parts, icount, ncount] for DoubleRow operands."""
        return AP(base_ap.tensor, base_ap.offset + offset,
                  [[base_ap.ap[0][0], parts],
                   [istride, icount], [nstride, ncount]])

    with tile.TileContext(nc) as tc, ExitStack() as st0:
        consts = st0.enter_context(tc.tile_pool(name='consts', bufs=1))
        h1t_pool = st0.enter_context(tc.tile_pool(name='h1t', bufs=1))
        pad_pool = st0.enter_context(tc.tile_pool(name='pads', bufs=1))
        outb_pool = st0.enter_context(tc.tile_pool(name='outb', bufs=3))
        oh_pool = st0.enter_context(tc.tile_pool(name='oh', bufs=3))

        w1_t = consts.tile([128, 256], fp8, tag='w1')
        w2_t = consts.tile([128, 2432], fp8, tag='w2')
        w3_t = consts.tile([128, 2432], fp8, tag='w3')
        b1_t = consts.tile([1, 2048], fp8, tag='b1')
        b2_t = consts.tile([128, 1], f32, tag='b2')
        b3_t = consts.tile([128, 1], f32, tag='b3')
        nc.scalar.dma_start(out=b1_t[:], in_=b1_d.ap())
        zero_t = consts.tile([128, 512], bf16, tag='zeros')
        nc.vector.memset(zero_t[:], 0.0)

        # chunk slot NBLK is an all-zero block: leftover DHT singles pair
        # against it so every DHT matmul runs in DoubleRow mode
        h1T = h1t_pool.tile([128, (NBLK + 1) * 128], fp8, tag='h1T')
        nc.vector.memset(h1T[:, NBLK * 128:(NBLK + 1) * 128], 0.0)

        oh_tiles = {}

        def issue_oh(b, eng):
            t = oh_pool.tile([128, OHMAX], fp8, tag='oh', name=f'oh_{b}')
            cols = BAND_OFF[b + 1] - BAND_OFF[b]
            eng.dma_start(out=t[:BP, :cols],
                          in_=oh_d.ap()[0:BP, BAND_OFF[b]:BAND_OFF[b + 1]])
            oh_tiles[b] = t

        # conv inputs as fp8 hi/lo pairs in ONE tile each ([lo | hi] halves
        # so all DoubleRow pair strides stay positive); h2_pad keeps the
        # bf16 conv2 output so lo = relu(psum+b) - hi is computable.
        HOFF = PADW * PADW
        dpad8 = pad_pool.tile([128, 2 * HOFF], fp8, tag='dpad8')
        hpad8 = pad_pool.tile([128, 2 * HOFF], fp8, tag='hpad8')
        h2_pad = pad_pool.tile([128, PADW * PADW], bf16, tag='h2_pad')

        def pad_border_memsets():
            # zero only the borders; the interior is fully overwritten.
            for pad_t in (dpad8, hpad8):
                pv = pad_t[:].rearrange('c (two a r) -> c two a r', two=2,
                                        a=PADW)
                nc.gpsimd.memset(pv[:, :, 0:1, :], 0.0)
                nc.gpsimd.memset(pv[:, :, PADW - 1:PADW, :], 0.0)
                nc.gpsimd.memset(pv[:, :, :, 0:1], 0.0)
                nc.gpsimd.memset(pv[:, :, :, PADW - 1:PADW], 0.0)

        with ExitStack() as stT:
            # -------------------------------------------- transposed conv1
            # x arrives pixel-blocked ([ci, chunk*100+p]) so each chunk is a
            # contiguous 100-col slab.  conv1 runs TRANSPOSED: h1T[p, co] =
            # x_chunk^T @ w1 (x stationary, weights moving), writing h1T
            # directly -- no PE transposes, no psum drains, no h1 buffer.
            # Bias rides in as a K=1 DoubleRow product (ones x [bhi|blo]).
            with ExitStack() as st1:
                xf_pool = st1.enter_context(tc.tile_pool(name='xf', bufs=2))
                xb_pool = st1.enter_context(tc.tile_pool(name='xb', bufs=1))
                ps1 = st1.enter_context(
                    tc.tile_pool(name='ps1', bufs=3, space='PSUM'))

                GW = 10 * 228              # 2280: one block-row of chunks
                xbig = xb_pool.tile([128, 8 * GW], fp8, tag='xbig')
                ones_t = consts.tile([1, 256], fp8, tag='ones')
                nc.vector.memset(ones_t[:], 1.0)

                xf_tiles = {}

                def x_lhsT(k):
                    """stationary x chunk [K=128, 2(half), 100(pix)]."""
                    g = k // 10
                    kk = k % 10
                    if g < 2:
                        xa = xf_tiles[g][:]
                        off = kk * 228
                    else:
                        xa = xbig[:]
                        off = (g - 2) * GW + kk * 228
                    return AP(xa.tensor, xa.offset + off,
                              [[xa.ap[0][0], 128], [128, 2], [1, 100]])

                wp_t = []

                def warmup():
                    # PE p-state warm-up: ~3.2us of back-to-back dummy
                    # matmuls in the otherwise-idle window while the first
                    # x DMAs land, so real work starts at full clock
                    wp = ps1.tile([128, 1024], f32, tag='ps1', name='warm')
                    wp_t.append(wp)
                    for i in range(5):
                        nc.tensor.matmul(out=wp[:, 0:512],
                                         lhsT=zero_t[:1, :128],
                                         rhs=zero_t[:1, :512],
                                         start=True, stop=True,
                                         skip_group_check=True)

                def conv1_bank(b):
                    """8 chunks (4 for the tail bank) -> h1T fp8."""
                    k0 = b * 8
                    kc = min(8, NBLK - k0)
                    ps = ps1.tile([128, 1024], f32, tag='ps1',
                                  name=f'c1_{b}')
                    ba = b1_t[:]
                    oa = ones_t[:]
                    # bias init: one K=1 DoubleRow matmul per 512-col zero
                    # region (adds bhi + blo exactly)
                    for rg in range((kc + 3) // 4):
                        n = min(512, kc * 128 - rg * 512)
                        lhsT = AP(oa.tensor, oa.offset,
                                  [[oa.ap[0][0], 1], [128, 2], [1, 128]])
                        rhs = AP(ba.tensor, ba.offset + rg * 512,
                                 [[ba.ap[0][0], 1], [1024, 2], [1, n]])
                        nc.tensor.matmul(out=ps[:, rg * 512:rg * 512 + n],
                                         lhsT=lhsT, rhs=rhs, start=True,
                                         stop=False, perf_mode=DR,
                                         skip_group_check=True)
                    for kk in range(kc):
                        k = k0 + kk
                        rhs = dr_ap(w1_t[:], 0, 128, 2, 1, 128, 128)
                        nc.tensor.matmul(
                            out=ps[:BP, kk * 128:(kk + 1) * 128],
                            lhsT=x_lhsT(k), rhs=rhs, start=False,
                            stop=(kk in (3, kc - 1)), perf_mode=DR,
                            skip_group_check=True)
                    # epilogue: relu * SC1, psum -> h1T fp8
                    dst = h1T[:BP, k0 * 128:(k0 + kc) * 128]
                    src = ps[:BP, :kc * 128]
                    if b % 2 == 0:
                        nc.scalar.activation(out=dst, in_=src, func=RELU,
                                             scale=SC1)
                    else:
                        nc.vector.tensor_scalar(
                            out=dst, in0=src, scalar1=0.0, scalar2=SC1,
                            op0=mybir.AluOpType.max,
                            op1=mybir.AluOpType.mult)

                for g in range(2):
                    xf = xf_pool.tile([128, GW], fp8, tag='xf',
                                      name=f'xf_{g}')
                    xf_tiles[g] = xf
                    (nc.sync, nc.gpsimd)[g].dma_start(
                        out=xf[:], in_=x_d.ap()[:, g * GW:(g + 1) * GW])
                    if g == 0:
                        nc.sync.dma_start(out=w1_t[:], in_=w1_d.ap())
                    if g == 1:
                        # groups 2-9 as 2-group DMAs (balances HWDGE fixed
                        # cost against DMA-device hold time)
                        for gg in range(4):
                            (nc.sync, nc.gpsimd)[gg % 2].dma_start(
                                out=xbig[:, gg * 2 * GW:(gg + 1) * 2 * GW],
                                in_=x_d.ap()[:, (2 + 2 * gg) * GW:
                                             (4 + 2 * gg) * GW])
                # one-hot bands 0-2 + conv weights on the GPSIMD queue,
                # behind its x pieces: SWDGE enqueue order keeps their
                # DMA-device FIFO slots behind the whole x stream
                issue_oh(0, nc.gpsimd)
                issue_oh(1, nc.gpsimd)
                issue_oh(2, nc.gpsimd)
                nc.gpsimd.dma_start(out=w2_t[:], in_=w2_d.ap())
                nc.gpsimd.dma_start(out=w3_t[:], in_=w3_d.ap())
                warmup()
                for b in range(13):
                    conv1_bank(b)

            pad_border_memsets()
            nc.scalar.dma_start(out=b2_t[:], in_=b2_d.ap())
            nc.scalar.dma_start(out=b3_t[:], in_=b3_d.ap())

            psd = stT.enter_context(
                tc.tile_pool(name='psd', bufs=4, space='PSUM'))
            psc = stT.enter_context(
                tc.tile_pool(name='psc', bufs=2, space='PSUM'))

            # -------------------------------------------------- DHT + convs
            def zero_bank(b, bank):
                # initialize the accumulator: all DHT matmuls use start=False
                # and accumulate onto zeroed PSUM.
                if b < 1:
                    nc.tensor.matmul(out=bank[:, :BAND * R],
                                     lhsT=zero_t[:1, :128],
                                     rhs=zero_t[:1, :BAND * R], start=True,
                                     stop=False, skip_group_check=True)
                elif b == 1:
                    nc.scalar.activation(out=bank[:, :BAND * R],
                                         in_=zero_t[:, :BAND * R],
                                         func=COPY)
                else:
                    nc.vector.memset(bank[:, :BAND * R], 0.0)

            def dht_band(b, bank):
                oh = oh_tiles[b]
                off0 = BAND_OFF[b]
                for s in range(BAND):
                    a = b * BAND + s
                    for _, k1, k2, lo_u, win_u, coloff in ENTRIES[a]:
                        lhsT = dr_ap(h1T[:], k1 * 128,
                                     (k2 - k1) * 128, 2, 1, 128, BP)
                        rhs = dr_ap(oh[:], coloff - off0,
                                    win_u, 2, 1, win_u, BP)
                        nc.tensor.matmul(
                            out=bank[:, s * R + lo_u:
                                     s * R + lo_u + win_u],
                            lhsT=lhsT, rhs=rhs, start=False, stop=False,
                            skip_group_check=True, perf_mode=DR)

            def pad_views(pad_t, a0, na):
                """(lo, hi) interior views [c, na, R] of a hi/lo pad tile."""
                pv = pad_t[:].rearrange('c (two a r) -> c two a r', two=2,
                                        a=PADW)
                lo = pv[:, 0:1, a0 + 1:a0 + 1 + na, 1:1 + R].squeeze(1)
                hi = pv[:, 1:2, a0 + 1:a0 + 1 + na, 1:1 + R].squeeze(1)
                return lo, hi

            def drain_band(b, bank):
                # psum -> fp8 hi (ACT) + fp8 lo residual (DVE subtract)
                a0 = b * BAND
                pv = bank[:, :BAND * R].rearrange('p (a r) -> p a r', a=BAND)
                lo_v, hi_v = pad_views(dpad8, a0, BAND)
                nc.scalar.activation(out=hi_v, in_=pv[:], func=COPY)
                nc.vector.tensor_tensor(out=lo_v, in0=pv[:], in1=hi_v,
                                        op=mybir.AluOpType.subtract)

            # conv psum layout: per band one [128, 1024] tile = 2 zero
            # regions; sub-band s2 (2 angles) accumulates over the FLATTENED
            # padded rows at cols [s2*512, s2*512+202) (junk at row-boundary
            # cols, skipped by the epilogue's PADW-strided reads).
            LSUB = PADW + R    # 202

            def emit14(ps, s2, pad_t, w_t, a0, lsub=None):
                """3-product fp8 conv: 14 DoubleRow passes accumulating
                w_hi*x_hi + w_hi*x_lo + w_lo*x_hi over 9 taps."""
                def wc(g, t):
                    return 128 + g * 1152 + t * 128

                def xc(h, t):
                    return (h * HOFF + (t // 3) * PADW + (t % 3)
                            + a0 * PADW)

                pairs = []
                for t in (0, 2, 4, 6):
                    pairs.append(((wc(0, t), xc(1, t)),
                                  (wc(0, t + 1), xc(1, t + 1))))
                for t in (0, 2, 4, 6):
                    pairs.append(((wc(0, t), xc(0, t)),
                                  (wc(0, t + 1), xc(0, t + 1))))
                for t in (0, 2, 4, 6):
                    pairs.append(((wc(1, t), xc(1, t)),
                                  (wc(1, t + 1), xc(1, t + 1))))
                pairs.append(((wc(0, 8), xc(0, 8)), (wc(1, 8), xc(1, 8))))
                # odd 27th product pairs with the zero-weight block (w col
                # 0); the dummy rhs member re-reads tap-0's valid window
                pairs.append(((0, xc(1, 0)), (wc(0, 8), xc(1, 8))))
                wa = w_t[:]
                xa = pad_t[:]
                if lsub is None:
                    lsub = LSUB
                out_v = ps[:, s2 * 512:s2 * 512 + lsub]
                for i, ((w1c, x1c), (w2c, x2c)) in enumerate(pairs):
                    lhsT = AP(wa.tensor, wa.offset + w1c,
                              [[wa.ap[0][0], 128], [w2c - w1c, 2], [1, 128]])
                    rhs = AP(xa.tensor, xa.offset + x1c,
                             [[xa.ap[0][0], 128], [x2c - x1c, 2], [1, lsub]])
                    nc.tensor.matmul(out=out_v, lhsT=lhsT, rhs=rhs,
                                     start=(i == 0),
                                     stop=(i == len(pairs) - 1),
                                     perf_mode=DR)

            def conv_psum_view(ps, ar, sub=0):
                """[p, (sub, al), R] view of the padded conv psum."""
                pa = ps[:]
                if ar == BAND:
                    return AP(pa.tensor, pa.offset,
                              [[pa.ap[0][0], 128], [512, 2], [PADW, 2],
                               [1, R]])
                if ar == 1:
                    return AP(pa.tensor, pa.offset + sub * 512,
                              [[pa.ap[0][0], 128], [1, R]])
                return AP(pa.tensor, pa.offset,
                          [[pa.ap[0][0], 128], [PADW, ar], [1, R]])

            def conv2_band(c):
                a0 = c * BAND
                ps = psc.tile([128, 1024], f32, tag='conv')
                for s2 in range(2):
                    emit14(ps, s2, dpad8, w2_t, a0 + s2 * 2)
                pv = conv_psum_view(ps, BAND)
                hv2 = h2_pad[:].rearrange('c (a r) -> c a r', a=PADW)
                h2v = hv2[:, a0 + 1:a0 + 1 + BAND, 1:1 + R]
                h2v4 = h2v.rearrange('c (x y) r -> c x y r', x=2)
                nc.scalar.activation(out=h2v4, in_=pv, func=RELU,
                                     bias=b2_t[:, :1], scale=SC2)
                lo_v, hi_v = pad_views(hpad8, a0, BAND)
                nc.scalar.activation(out=hi_v, in_=h2v, func=COPY)
                nc.vector.tensor_tensor(out=lo_v, in0=h2v, in1=hi_v,
                                        op=mybir.AluOpType.subtract)

            def conv3_band(c, ar=BAND, s0=0, dve_epi=False, ps=None, sub=0):
                a0 = c * BAND + s0
                if ps is None:
                    ps = psc.tile([128, 1024], f32, tag='conv')
                if ar == 1:
                    emit14(ps, sub, hpad8, w3_t, a0, lsub=R)
                else:
                    for s2 in range(ar // 2):
                        emit14(ps, s2, hpad8, w3_t, a0 + s2 * 2)
                pv = conv_psum_view(ps, ar, sub)
                ob = outb_pool.tile([128, ar * R], f32, tag=f'outb{ar}')
                if ar == BAND:
                    ov = ob[:].rearrange('p (x y r) -> p x y r', x=2, y=2)
                elif ar == 1:
                    ov = ob[:]
                else:
                    ov = ob[:].rearrange('p (a r) -> p a r', a=ar)
                if dve_epi:
                    nc.vector.tensor_scalar(
                        out=ov, in0=pv, scalar1=b3_t[:, :1],
                        scalar2=0.0, op0=mybir.AluOpType.add,
                        op1=mybir.AluOpType.max)
                else:
                    nc.scalar.activation(out=ov, in_=pv, func=RELU,
                                         bias=b3_t[:, :1], scale=1.0)
                nc.sync.dma_start(out=out_d.ap()[:, a0 * R:(a0 + ar) * R],
                                  in_=ob[:])

            banks = {0: psd.tile([128, 512], f32, tag='band',
                                 name='bank_0')}
            zero_bank(0, banks[0])
            for b in range(NBAND):
                # stream the upcoming one-hot bands behind the PE
                if b + 3 < NBAND:
                    issue_oh(b + 3, (nc.sync, nc.gpsimd, nc.scalar)[b % 3])
                if b + 1 < NBAND:
                    banks[b + 1] = psd.tile([128, 512], f32, tag='band',
                                            name=f'bank_{b + 1}')
                    zero_bank(b + 1, banks[b + 1])
                dht_band(b, banks[b])
                drain_band(b, banks[b])
                del banks[b]
                del oh_tiles[b]
                if b >= 2:
                    conv2_band(b - 2)
                if b >= 4:
                    conv3_band(b - 4)
            for c in (NBAND - 2, NBAND - 1):
                conv2_band(c)
            for c in range(NBAND - 4, NBAND - 1):
                conv3_band(c)
            # split the last band so its epilogue/DMA pipeline with the
            # later pieces' matmuls instead of trailing the whole kernel
            conv3_band(NBAND - 1, ar=2, s0=0)
            ps_tail = psc.tile([128, 1024], f32, tag='conv')
            conv3_band(NBAND - 1, ar=1, s0=2, ps=ps_tail, sub=0)
            conv3_band(NBAND - 1, ar=1, s0=3, dve_epi=True, ps=ps_tail,
                       sub=1)

    if split_waits:
        _split_multi_waits(nc, mybir)
    _PROGRAM_CACHE[key] = nc
    return nc


# ----------------------------------------------------------------------------
# entry point
# ----------------------------------------------------------------------------

def make_in_maps(inputs):
    plan = _dht_plan()
    x = np.asarray(inputs['x'], np.float32)
    w1p8, bias18, w2p8, bias2, w3p8, bias3 = _prep_weights(
        *[np.asarray(inputs[k], np.float32) for k in
          ('w1', 'b1', 'g1', 'be1', 'm1', 'v1',
           'w2', 'b2', 'g2', 'be2', 'm2', 'v2',
           'w3', 'b3', 'g3', 'be3', 'm3', 'v3')])
    common = dict(w1p8=w1p8, bias18=bias18, w2p8=w2p8, bias2=bias2,
                  w3p8=w3p8, bias3=bias3, ohtab=plan['ohtab'])
    # x host-permuted: [ci%128, chunk*228 + (ci//128)*128 + p] with
    # p = dy*10+dx, chunk = gy*10+gx; 28-byte zero gaps per chunk
    xb = (x.reshape(N, 2, 128, YBLK, BH, XBLK, BW)
          .transpose(0, 2, 3, 5, 1, 4, 6)      # n, ci128, gy, gx, half, dy, dx
          .reshape(N, 128, NBLK, 2, BP))
    xp = np.zeros((N, 128, NBLK, 228), FP8)
    xp[:, :, :, 0:100] = xb[:, :, :, 0].astype(FP8)
    xp[:, :, :, 128:228] = xb[:, :, :, 1].astype(FP8)
    xp = xp.reshape(N, 128, NBLK * 228)
    return [
        {'x': np.ascontiguousarray(xp[n]), **common}
        for n in range(N)
    ]


def run(inputs, trace=False):
    from concourse.bass_utils import run_bass_kernel_spmd

    nc = _build_program()
    in_maps = make_in_maps(inputs)
    res = run_bass_kernel_spmd(nc, in_maps, core_ids=list(range(N)),
                               trace=trace)
    out = np.stack([res.results[n]['out'].reshape(CMID, H, W)
                    for n in range(N)], axis=0)
    return out.astype(np.float32), res


def kernel(**inputs):
    out, _ = run(inputs, trace=False)
    return out
